# revision 48
# baseline (speedup 1.0000x reference)
"""Trainium2 Bass kernel for nn_BKCoreHyperbolicIntegration (8 NeuronCores).

Reference computation:
    he[b,s]  = mean_e( x[b,s,:] @ Wd[e,:] + bd[e] ) = x @ colmean(Wd) + mean(bd)
    G        = 1 / (he - (0 + 0.1j) + 1e-6)
    gate     = sigmoid(gW[0,0]*Re(G) + gW[0,1]*Im(G) + gb[0])
    gated    = attention_weights * gate[:, None, :, None]
    out      = gated / (gated.sum(-1, keepdims=True) + 1e-6)

Algebra used (all exact, no approximation):
  * mean_e(x @ Wd.T + bd) == x @ colmean(Wd) + mean(bd).
  * h0_super / h0_sub in the reference are dead code -> skipped.
  * With z = 0.1j and d := he + EPS:  Re G = d/(d^2+0.01), Im G = 0.1/(d^2+0.01).
  * out = attn*g / (g*rs + EPS) == attn / (rs + EPS/g), and for g = sigmoid(z),
    1/g = 1 + exp(-z).  So the gate enters only through the tiny per-row
    denominator bias cb = EPS*(1 + exp(-z)); no sigmoid and no second
    gate multiply are needed.

Sharding: the S (row) axis of attention_weights is split across the 8 cores
(core k owns rows [128k, 128k+128) for every b,h).  Wd is COLUMN-sharded:
core k loads Wd[:, 256k:256k+256) (host-relaid into PE-friendly tiles),
PE-reduces it to its exact colsum slice [1,256], and an 8-core AllGather
(no reduce pass, so ~half the modeled cost of AllReduce) assembles the full
[1,2048] column sum, which is broadcast per-partition for the DVE dot
product with x.

All bulk tensors are host-relaid so every DMA moves contiguous 8-16KB runs
per partition (128 descriptors) - descriptor-count overhead on the shared
DMA engines is ~1ns/descriptor, so small descriptors cost real time.

The attention stream uses 16 pieces of [128, 2*1024] (2 heads per piece),
all resident in SBUF (no ring reuse).  DMA queues serialize per engine
(~transfer rate each) but overlap across engines, so both the in- and the
out-stream are spread over three queues (SP / ACT / Pool), with piece
ownership chosen so each reduce's input has landed before its engine
needs it and the Pool queue is clear when cc_in / wbar need it.

Per piece: a row-sum reduce (DVE reduce_sum or ACT Copy+accum_out), then
ACT folds den = rs + cb into Ln's bias, Exp gives the reciprocal, DVE
bounces it into sc_all (so the scale-port operand of the ACT muls comes
from a different engine), DVE scales head g=0 and ACT head g=1 in place
(software-pipelined: batch q's chain overlaps batch q-1's muls), and the
piece is DMA'd out from whichever queue owns it.

Toolchain behaviors inherited from the validated baseline:
  * all semaphores explicit, 1-2 waits per instruction (compiler limit).
  * reciprocal = exp(-ln(x)) on ACT (InstReciprocal returns inf on HW).
  * same-engine dependent pairs completion-synced via chain semaphores;
    scalar-port operands produced by a different engine.
  * DMA completion quanta: [128,*] DMAs post 16, [1,*] post 32.
  * PE is warmed with dummy matmuls until wd arrives so the colsum runs
    at full clock (the pstate ramp resets if the PE goes idle).

Engine roles:
  SP     wd/x prefix + 6 attention ins, 7 outs
  PE     warmup + Wd column-sum (ones.T @ wd tiles, PSUM-accumulated)
  Pool   4 early ins, cc_in, AllGather, broadcasts, extras, 7 outs
  DVE    10 reduces, he multiplies, gate chain, sc bounce, g=0 muls
  ACT    6 ins, bd/he accumulation, Ln/Exp chains, g=1 muls, 2 last outs
"""

from contextlib import ExitStack

import numpy as np

import concourse.bass as bass
from concourse import mybir
from concourse.bass_utils import run_bass_kernel_spmd

TRACE = False
LAST_EXEC_NS = None
LAST_RESULTS = None

F32 = mybir.dt.float32
AX = mybir.AxisListType
ALU = mybir.AluOpType
ACT_F = mybir.ActivationFunctionType

B, S, H, D = 2, 1024, 16, 2048
N_CORES = 8
S_CHUNK = S // N_CORES        # 128 attn rows per core
BH = B * H                    # 32
DSL = D // N_CORES            # 256 Wd columns per core
NWT = D // 128                # 16 row-tiles of the Wd column slice
PG = 2                        # heads per attention piece
NP = BH // PG                 # 16 pieces
NCH = NP // 2                 # 8 ACT chain batches ([128, 4] each)
EPS = 1e-6
INV_D = 1.0 / D
Q_IN = 16                     # [128,*] DMA completion quantum
Q_CC = 32                     # [1,*] DMA completion quantum
THROTTLE = 3                  # max in-flight attention in-DMAs
N_HOIST = 10                  # reduces hoisted before the gate chain


def build_kernel(debug: bool = False, detect_races: bool = True):
    nc = bass.Bass(detect_race_conditions=detect_races)
    attn_in = nc.declare_dram_parameter("attn", [NP, 128, PG * S], F32, isOutput=False)
    xs_in = nc.declare_dram_parameter("xs", [128, B * D], F32, isOutput=False)
    wdc_in = nc.declare_dram_parameter("wdc", [128, NWT * DSL], F32, isOutput=False)
    bd_in = nc.declare_dram_parameter("bd", [1, D], F32, isOutput=False)
    gwb_in = nc.declare_dram_parameter("gwb", [1, 3], F32, isOutput=False)
    out_d = nc.declare_dram_parameter("out", [NP, 128, PG * S], F32, isOutput=True)
    cc_in = nc.dram_tensor("cc_in", [1, DSL], F32)
    cc_out = nc.dram_tensor("cc_out", [1, D], F32, addr_space="Shared")
    extras_dram = nc.dram_tensor("extras_dram", [1, 4], F32)

    ctx = ExitStack()
    with ctx:
        sb = lambda shape, name: ctx.enter_context(
            nc.sbuf_tensor(name, shape, F32))
        sem = lambda name: ctx.enter_context(nc.semaphore(name))

        tin = [sb([128, PG * S], f"tin{i}") for i in range(NP)]
        wd_sb = sb([128, NWT * DSL], "wd_sb")
        xt = sb([128, B * D], "xt")
        wbar_sb = sb([128, D], "wbar_sb")
        bd_sb = sb([1, D], "bd_sb")
        gwb_sb = sb([1, 3], "gwb_sb")
        cs_sb = sb([1, DSL], "cs_sb")
        cc_sb = sb([1, D], "cc_sb")
        dinit = sb([1, 1], "dinit")
        staging = sb([1, 4], "staging")
        extras_sb = sb([128, 4], "extras_sb")
        rs_all = sb([128, BH], "rs_all")
        rec_scr = sb([128, BH], "rec_scr")
        rec_all = sb([128, BH], "rec_all")
        sc_all = sb([128, BH], "sc_all")
        ghraw = sb([128, B], "ghraw")
        dcol = sb([128, B], "dcol")
        gden = sb([128, B], "gden")
        grscr = sb([128, B], "grscr")
        grec = sb([128, B], "grec")
        gt1 = sb([128, B], "gt1")
        gt1g = sb([128, B], "gt1g")
        gt2g = sb([128, B], "gt2g")
        etm = sb([128, B], "etm")
        cb = sb([128, B], "cb")
        ones_col = sb([128, 1], "ones_col")
        c001 = sb([128, 1], "c001")
        warm = sb([128, 512], "warm")
        colsum_ps = ctx.enter_context(nc.psum_tensor("colsum_ps", [1, DSL], F32))
        warm_ps = ctx.enter_context(nc.psum_tensor("warm_ps", [1, 512], F32))

        s_in = [sem(f"s_in{i}") for i in range(NP)]
        s_x = sem("s_x")
        s_wdq = [sem(f"s_wdq{i}") for i in range(4)]
        s_rs_d = sem("s_rs_d")
        s_rs_a = sem("s_rs_a")
        s_md = sem("s_md")
        s_rs_p = sem("s_rs_p")
        s_cci = sem("s_cci")
        s_cc = sem("s_cc")
        s_ws = sem("s_ws")
        s_exo = sem("s_exo")
        s_exb = sem("s_exb")
        s_ones = sem("s_ones")
        s_pe = sem("s_pe")
        s_colsum_sb = sem("s_colsum_sb")
        s_dinit = sem("s_dinit")
        s_staging = sem("s_staging")
        s_hemul = sem("s_hemul")
        s_ghr = sem("s_ghr")
        s_gden = sem("s_gden")
        s_grec = sem("s_grec")
        s_lin = sem("s_lin")
        s_et = sem("s_et")
        s_cb = sem("s_cb")
        s_rs = sem("s_rs")
        s_recact = sem("s_recact")
        s_sc = sem("s_sc")
        s_ma = sem("s_ma")
        s_vchain = sem("s_vchain")
        s_achain = sem("s_achain")
        s_sink = sem("s_sink")
        s_out = sem("s_out")

        with nc.Block() as block:

            # piece ownership: in-queues, reduce engines, out-queues
            IN_ACT = [0, 1, 2, 3, 4, 5]         # ACT HWDGE queue
            IN_SP = [10, 11, 6, 7, 8, 9]        # SP queue; 10/11 first (DVE needs them pre-he)
            IN_POOL = [12, 13, 14, 15]          # Pool queue, before cc_in
            RED_D = [12, 13, 14, 0, 1, 2, 10, 11, 8, 9]  # DVE, arrival order
            RED_A = [15, 3, 4, 5, 6, 7]                  # ACT accum reduces
            OUT_SP = [0, 2, 4, 6, 8, 10, 12]
            OUT_POOL = [1, 3, 5, 7, 9, 11, 13]
            OUT_ACT = [14, 15]                  # after ACT's last muls
            HWT = NWT // 2

            def chain_waits(eng, q):
                """Reduce-stream counts each chain batch q must wait for."""
                need = (2 * q, 2 * q + 1)
                order = {"d": RED_D, "a": RED_A}[eng]
                return max([order.index(p) + 1 for p in need if p in order],
                           default=0)

            @block.sync
            def _(sync):
                # wd first: it gates colsum -> AllGather -> gate.  bd/gwb
                # have no direct waiters - covered via queue FIFO by in6.
                for wq in range(4):
                    c0, c1 = wq * 4 * DSL, (wq + 1) * 4 * DSL
                    sync.dma_start(
                        wd_sb[:, c0:c1], wdc_in[:, c0:c1]
                    ).then_inc(s_wdq[wq], 16)
                sync.dma_start(xt[:], xs_in[:]).then_inc(s_x, 16)
                sync.dma_start(bd_sb[:], bd_in[:]).then_inc(s_sink, 16)
                sync.dma_start(gwb_sb[:], gwb_in[:]).then_inc(s_sink, 16)
                for p in IN_SP:
                    sync.dma_start(
                        tin[p][:], attn_in[p]).then_inc(s_in[p], Q_IN)
                for p in OUT_SP:
                    sync.wait_ge(s_ma, p + 1)
                    sync.wait_ge(s_md, p + 1)
                    sync.dma_start(out_d[p], tin[p][:]).then_inc(s_sink, 16)

            @block.tensor
            def _(tensor):
                # warm the PE clock (pstate ramps with continuous busy time)
                tensor.wait_ge(s_ones, 1)
                for _ in range(6):
                    nc.tensor.matmul(
                        warm_ps[:], lhsT=ones_col[:], rhs=warm[:],
                        start=True, stop=True)
                for t in range(NWT):
                    if t % 4 == 0:
                        tensor.wait_ge(s_wdq[t // 4], 16)
                    mm = nc.tensor.matmul(
                        colsum_ps[:],
                        lhsT=ones_col[:],
                        rhs=wd_sb[:, t * DSL:(t + 1) * DSL],
                        start=(t == 0), stop=(t == NWT - 1))
                mm.then_inc(s_pe, 1)

            @block.gpsimd
            def _(gpsimd):
                for p in IN_POOL:
                    gpsimd.dma_start(
                        tin[p][:], attn_in[p]).then_inc(s_in[p], Q_IN)
                gpsimd.wait_ge(s_colsum_sb, 1)
                gpsimd.dma_start(cc_in[:], cs_sb[:]).then_inc(s_cci, Q_CC)
                gpsimd.wait_ge(s_cci, Q_CC)
                gpsimd.collective_compute(
                    "AllGather",
                    ALU.bypass,
                    replica_groups=[list(range(N_CORES))],
                    ins=[cc_in[:]],
                    outs=[cc_out[:]],
                ).then_inc(s_cc, 1)
                gpsimd.wait_ge(s_cc, 1)
                gpsimd.dma_start(
                    wbar_sb[:], cc_out[:].broadcast_to((128, D))
                ).then_inc(s_ws, 16)
                gpsimd.wait_ge(s_staging, 1)
                gpsimd.dma_start(extras_dram[:], staging[:]).then_inc(s_exo, 16)
                gpsimd.wait_ge(s_exo, 16)
                gpsimd.dma_start(
                    extras_sb[:], extras_dram[:].broadcast_to((128, 4))
                ).then_inc(s_exb, 16)
                for p in OUT_POOL:
                    gpsimd.wait_ge(s_ma, p + 1)
                    gpsimd.wait_ge(s_md, p + 1)
                    gpsimd.dma_start(out_d[p], tin[p][:]).then_inc(s_out, 16)

            @block.vector
            def _(vector):
                vc = 0
                nc.vector.memset(warm[:], 1.0)
                nc.vector.memset(c001[:], 0.01)
                nc.vector.memset(ones_col[:], 1.0).then_inc(s_ones, 1)
                # row-sum reduces, ordered by expected piece arrival;
                # cs-copy / staging slot in between the early ones
                for i, p in enumerate(RED_D):
                    if i == 2:
                        # PE colsum lands about now; ship it to the AllGather
                        vector.wait_ge(s_pe, 1)
                        nc.vector.tensor_copy(
                            cs_sb[:], colsum_ps[:]).then_inc(s_colsum_sb, 1)
                    vector.wait_ge(s_in[p], Q_IN)
                    nc.vector.reduce_sum(
                        rs_all[:, p * PG:(p + 1) * PG],
                        tin[p].rearrange("p (g t) -> p g t", g=PG),
                        axis=AX.X).then_inc(s_rs_d, 1)
                # staging = [gW00, 0.1*gW01, -gb, mean(bd)+EPS]
                vector.wait_ge(s_in[10], Q_IN)  # bd+gwb (queue FIFO)
                vector.wait_ge(s_dinit, 1)
                nc.vector.tensor_copy(staging[:, 0:1], gwb_sb[:, 0:1])
                nc.vector.tensor_scalar(
                    out=staging[:, 1:2], in0=gwb_sb[:, 1:2],
                    scalar1=0.1, scalar2=None, op0=ALU.mult)
                nc.vector.tensor_scalar(
                    out=staging[:, 2:3], in0=gwb_sb[:, 2:3],
                    scalar1=-1.0, scalar2=None, op0=ALU.mult)
                nc.vector.tensor_copy(
                    staging[:, 3:4], dinit[:]).then_inc(s_staging, 1)
                # he multiplies (ACT accumulates them into ghraw)
                vector.wait_ge(s_ws, 16)
                vector.wait_ge(s_x, 16)
                for b in range(B):
                    nc.vector.tensor_mul(
                        xt[:, b * D:(b + 1) * D], xt[:, b * D:(b + 1) * D],
                        wbar_sb[:]).then_inc(s_hemul, 1)
                # gate chain; same-engine dependent pairs completion-synced
                vector.wait_ge(s_ghr, B)
                vector.wait_ge(s_exb, 16)
                nc.vector.tensor_scalar(
                    out=dcol[:], in0=ghraw[:],
                    scalar1=INV_D, scalar2=extras_sb[:, 3:4],
                    op0=ALU.mult, op1=ALU.add).then_inc(s_vchain, 1)
                vc += 1; vector.wait_ge(s_vchain, vc)
                nc.vector.tensor_scalar(
                    out=gt1[:], in0=dcol[:], scalar1=extras_sb[:, 0:1],
                    scalar2=extras_sb[:, 1:2], op0=ALU.mult, op1=ALU.add
                ).then_inc(s_vchain, 1)
                vc += 1; vector.wait_ge(s_vchain, vc)
                vector.wait_ge(s_grec, 1)
                nc.vector.tensor_mul(gt1g[:], gt1[:], grec[:]).then_inc(s_lin, 1)
                # cb = EPS * (1 + exp(-(lin+gb))) : the whole gate effect
                vector.wait_ge(s_et, 1)
                nc.vector.tensor_scalar(
                    out=cb[:], in0=etm[:], scalar1=EPS, scalar2=EPS,
                    op0=ALU.mult, op1=ALU.add).then_inc(s_cb, 1)
                # per batch: bounce sc (cross-engine scalar port), scale g=0
                for q in range(NCH + 1):
                    if q < NCH:
                        vector.wait_ge(s_recact, q + 1)
                        nc.vector.tensor_copy(
                            sc_all[:, q * 4:(q + 1) * 4],
                            rec_all[:, q * 4:(q + 1) * 4]).then_inc(s_sc, 1)
                        vector.wait_ge(s_sc, q + 1)
                    if q > 0:
                        for pp in range(2):
                            p = 2 * (q - 1) + pp
                            col = p * PG
                            nc.vector.tensor_scalar(
                                out=tin[p][:, 0:S], in0=tin[p][:, 0:S],
                                scalar1=sc_all[:, col:col + 1], scalar2=None,
                                op0=ALU.mult).then_inc(s_md, 1)

            @block.scalar
            def _(scalar):
                ac = 0
                for p in IN_ACT:
                    scalar.dma_start(
                        tin[p][:], attn_in[p]).then_inc(s_in[p], Q_IN)
                scalar.wait_ge(s_in[10], Q_IN)  # bd landed (queue FIFO)
                nc.scalar.activation(
                    bd_sb[:], bd_sb[:], ACT_F.Copy,
                    bias=EPS * INV_D, scale=INV_D, accum_out=dinit[:],
                ).then_inc(s_dinit, 1)
                # accum-reduces for the ACT-owned pieces
                for p in RED_A:
                    scalar.wait_ge(s_in[p], Q_IN)
                    for g in range(PG):
                        mi = nc.scalar.activation(
                            tin[p][:, g * S:(g + 1) * S],
                            tin[p][:, g * S:(g + 1) * S], ACT_F.Copy,
                            bias=0.0, scale=1.0,
                            accum_out=rs_all[:, p * PG + g:p * PG + g + 1])
                    mi.then_inc(s_rs_a, 1)
                # he accumulation: ghraw[:, b] = sum_d xt[:, b*D:(b+1)*D]
                for b in range(B):
                    scalar.wait_ge(s_hemul, b + 1)
                    nc.scalar.activation(
                        xt[:, b * D:(b + 1) * D], xt[:, b * D:(b + 1) * D],
                        ACT_F.Copy, bias=0.0, scale=1.0,
                        accum_out=ghraw[:, b:b + 1]).then_inc(s_ghr, 1)
                # grec = 1/(d^2+0.01), d = ghraw/D + dinit: Square runs
                # on ACT in parallel with DVE's linear-term leg
                scalar.wait_ge(s_ghr, B)
                scalar.wait_ge(s_exb, 16)
                nc.scalar.activation(
                    gden[:], ghraw[:], ACT_F.Square,
                    bias=extras_sb[:, 3:4], scale=INV_D).then_inc(s_achain, 1)
                ac += 1; scalar.wait_ge(s_achain, ac)
                nc.scalar.activation(
                    grscr[:], gden[:], ACT_F.Ln,
                    bias=c001[:], scale=1.0).then_inc(s_achain, 1)
                ac += 1; scalar.wait_ge(s_achain, ac)
                nc.scalar.activation(
                    grec[:], grscr[:], ACT_F.Exp,
                    bias=0.0, scale=-1.0).then_inc(s_grec, 1)
                # etm = exp(-(lin + gb))   (bias AP holds -gb)
                scalar.wait_ge(s_lin, 1)
                nc.scalar.activation(
                    etm[:], gt1g[:], ACT_F.Exp,
                    bias=extras_sb[:, 2:3], scale=-1.0).then_inc(s_et, 1)
                # per chain batch q (pieces 2q, 2q+1; 4 bh columns):
                #   rec = exp(-ln(rs + cb)) ; scale g=1 rows in place
                scalar.wait_ge(s_cb, 1)
                for q in range(NCH + 1):
                    if q < NCH:
                        b = q // (NCH // B)
                        cols = slice(q * 4, (q + 1) * 4)
                        for eng, sem_h in (("d", s_rs_d), ("a", s_rs_a)):
                            n = chain_waits(eng, q)
                            if n:
                                scalar.wait_ge(sem_h, n)
                        nc.scalar.activation(
                            rec_scr[:, cols], rs_all[:, cols], ACT_F.Ln,
                            bias=cb[:, b:b + 1], scale=1.0).then_inc(s_achain, 1)
                        ac += 1; scalar.wait_ge(s_achain, ac)
                        nc.scalar.activation(
                            rec_all[:, cols], rec_scr[:, cols], ACT_F.Exp,
                            bias=0.0, scale=-1.0).then_inc(s_recact, 1)
                    if q > 0:
                        for pp in range(2):
                            p = 2 * (q - 1) + pp
                            col = p * PG + 1
                            if pp == 0:
                                scalar.wait_ge(s_sc, q)
                            nc.scalar.activation(
                                tin[p][:, S:2 * S],
                                tin[p][:, S:2 * S], ACT_F.Copy,
                                bias=0.0, scale=sc_all[:, col:col + 1]
                            ).then_inc(s_ma, 1)
                # last two outs ride the ACT queue once its muls are done
                scalar.wait_ge(s_ma, NP)  # own g=1 muls complete
                for p in OUT_ACT:
                    scalar.wait_ge(s_md, p + 1)
                    scalar.dma_start(out_d[p], tin[p][:]).then_inc(s_sink, 16)
    return nc


_NC_CACHE = {}


def _get_nc():
    if "nc" not in _NC_CACHE:
        _NC_CACHE["nc"] = build_kernel()
    return _NC_CACHE["nc"]


def kernel(x, attention_weights, Wd, bd, Wsup, bsup, Wsub, bsub, gW, gb):
    """Full inputs in, full output out; shards internally across 8 cores."""
    global LAST_EXEC_NS, LAST_RESULTS
    x = np.ascontiguousarray(x, dtype=np.float32)
    attention_weights = np.ascontiguousarray(attention_weights, dtype=np.float32)
    Wd = np.ascontiguousarray(Wd, dtype=np.float32)
    bd_r = np.asarray(bd, dtype=np.float32).reshape(1, D)
    gwb = np.array([[np.float32(gW[0, 0]), np.float32(gW[0, 1]),
                     np.float32(gb[0])]], dtype=np.float32)

    nc = _get_nc()

    in_maps = []
    for k in range(N_CORES):
        sk = k * S_CHUNK
        ck = k * DSL
        # attn: [NP, 128, PG*S] with tin[p][s, g*S+t] = attn[bh=p*PG+g, s, t]
        attn_relay = np.ascontiguousarray(
            attention_weights[:, :, sk:sk + S_CHUNK, :]
            .reshape(NP, PG, S_CHUNK, S)
            .transpose(0, 2, 1, 3)
            .reshape(NP, 128, PG * S))
        # x: [128, B*D] with xt[s, b*D+d] = x[b, sk+s, d]
        xs_relay = np.ascontiguousarray(
            x[:, sk:sk + S_CHUNK, :].transpose(1, 0, 2).reshape(128, B * D))
        # wd: [128, NWT*DSL] with wd_sb[p, t*DSL+c] = Wd[t*128+p, ck+c]
        wd_relay = np.ascontiguousarray(
            Wd[:, ck:ck + DSL].reshape(NWT, 128, DSL)
            .transpose(1, 0, 2).reshape(128, NWT * DSL))
        in_maps.append({
            "attn": attn_relay,
            "xs": xs_relay,
            "wdc": wd_relay,
            "bd": bd_r,
            "gwb": gwb,
        })

    res = run_bass_kernel_spmd(nc, in_maps, list(range(N_CORES)), trace=TRACE)
    LAST_EXEC_NS = res.exec_time_ns
    LAST_RESULTS = res
    out = np.empty((B, H, S, S), dtype=np.float32)
    for k in range(N_CORES):
        sk = k * S_CHUNK
        out[:, :, sk:sk + S_CHUNK, :] = (
            res.results[k]["out"]
            .reshape(NP, S_CHUNK, PG, S)
            .transpose(0, 2, 1, 3)
            .reshape(B, H, S_CHUNK, S))
    return out
